# revision 31
# baseline (speedup 1.0000x reference)
"""FAPE loss kernel for Trainium2 (Bass/Tile), 8 NeuronCores.

Problem: B=8, N=1024.  reference computes, per batch b:
    R_i, t_i = backbone frames from (n, ca, c)          [N,3,3],[N,3]
    diff[i,j] = || R_i^T (pred_j - t_i) - R_i^T (true_j - t_i) ||
    per_pair  = min(diff,10) + 0.5*(diff - min(diff,10)) = 0.5*(diff + min(diff,10))
    out = sum_b sum_ij m_i m_j per_pair / (sum(m) + 1e-8)

Key reassociation (exact, no orthonormality assumption):
    R_i^T (pred_j - t_i) - R_i^T (true_j - t_i) = R_i^T d_j,  d_j = pred_j - true_j
    diff^2[i,j] = d_j^T (R_i R_i^T) d_j = sum_k q_k[j] * w_k[i]   (K=6)
  with q[j] = [d0^2, d1^2, d2^2, 2*d0d1, 2*d1d2, 2*d2d0] (masked by m_j)
       w[i] = [G00, G11, G22, G01, G12, G20], G = R_i R_i^T (masked by m_i)

Pairwise O(N^2) part per j-tile of 128:
  - TensorEngine matmul, K=18 bf16 hi/lo split (q = qh+ql, w = wh+wl bf16;
    rows [qh,qh,ql] x [wh,wl,wh] -> q.w exact up to ~2^-18) -> normsq PSUM
  - ACT sqrt PSUM->SBUF bf16
  - two DVE tensor_scalar passes at 4x bf16 rate: min(diff,10) and copy,
    each with accum_out giving the per-row sums
Per-core output is the [128, 8+8] accumulator sheet (sum_i per j-row for
diff and clamped); host sums sheets from the 8 cores and normalizes.

Frame-build latency tricks (all ~1e-7 relative, far below the ~2^-18 matmul
and bf16-diff rounding this kernel already carries):
  - z = normalize(cross(x, v)) == normalize(cross(c-ca, n-ca)) since the
    normalizations of x,v only scale the cross by a positive factor; this
    unserializes the z chain from the x chain.
  - y = cross(z, x) of two orthonormal unit vectors already has ||y|| = 1
    to ~1.3e-7, so its normalize is skipped.

Sharding: batch-parallel, one batch per core (spec hint allows B data-parallel).
"""

import numpy as np

P = 128          # partitions
T = 8            # j = 8*p + t  (p-major; any index bijection works for the sum)
N = 1024
B = 8
NCORES = 8

_cache: dict = {}


def _build_nc(diff_dtype="bf16", reps=0, prep_only=False):
    """Emit the single-core BIR module (same NEFF runs SPMD on all 8 cores)."""
    from contextlib import ExitStack

    import concourse.bacc as bacc
    import concourse.mybir as mybir
    import concourse.tile as tile
    from concourse import masks
    from concourse._compat import axon_active

    f32 = mybir.dt.float32
    bf16 = mybir.dt.bfloat16
    d_dt = bf16 if diff_dtype == "bf16" else f32
    Alu = mybir.AluOpType
    Act = mybir.ActivationFunctionType
    AxX = mybir.AxisListType.X

    nc = bacc.Bacc(
        "TRN2",
        target_bir_lowering=False,
        debug=not axon_active(),
        num_devices=NCORES,
    )

    # One concatenated input: cols [n(3) c(3) ca(3) pred(3) true(3) mask(1)]
    d_all = nc.dram_tensor("all_in", [N, 16], f32, kind="ExternalInput")
    d_out = nc.dram_tensor("out_acc", [P, 2 * T], f32, kind="ExternalOutput")

    with tile.TileContext(nc) as tc, ExitStack() as ctx:
        sb = ctx.enter_context(tc.tile_pool(name="sb", bufs=1))
        ps_t = ctx.enter_context(tc.tile_pool(name="ps_t", bufs=2, space="PSUM"))
        ps_ns = ctx.enter_context(tc.tile_pool(name="ps_ns", bufs=2, space="PSUM"))
        dpool = ctx.enter_context(tc.tile_pool(name="dpool", bufs=3))
        spool = ctx.enter_context(tc.tile_pool(name="spool", bufs=3))

        # ---- ACT table warmup: force the sqrt set load early (overlaps DMA)
        warm = sb.tile([1, 2], f32)
        nc.vector.memset(warm[:], 1.0)
        nc.scalar.activation(warm[:, 1:2], warm[:, 0:1], Act.Sqrt)

        # ---- ONE input DMA: [1024,16] -> [128, 8, 16], j = 8*p + t.
        # Fully contiguous in DRAM, 512B per partition.  Issued first.
        stg = sb.tile([P, T, 16], f32)
        nc.sync.dma_start(stg[:], d_all.ap().rearrange("(p t) c -> p t c", p=P))

        ident = sb.tile([P, P], bf16)
        masks.make_identity(nc, ident[:])

        rep_ctx = tc.For_i(0, reps, 1) if reps else None
        if rep_ctx is not None:
            rep_ctx.__enter__()
        t_nc2 = stg[:, :, 0:6].rearrange("p t (a c) -> p t a c", a=2)
        t_ca1 = stg[:, :, 6:9]
        t_pred = stg[:, :, 9:12]
        t_true = stg[:, :, 12:15]
        mask_bc6 = stg[:, :, 15:16].broadcast_to([P, T, 6])

        # ---- helpers ------------------------------------------------------
        def replicate(vec, name, eng=None):
            """[128,8,3] view -> [128,8,6] with r2[:, t, c] = vec[:, t, c % 3]."""
            r2 = sb.tile([P, T, 6], f32, tag=name)
            (eng or nc.vector).tensor_copy(
                r2[:].rearrange("p t (r c) -> p t r c", r=2),
                vec.unsqueeze(2).broadcast_to([P, T, 2, 3]),
            )
            return r2

        def cross(a2, b2, name, out=None):
            m1 = sb.tile([P, T, 3], f32, tag=f"{name}_m1")
            m2 = sb.tile([P, T, 3], f32, tag=f"{name}_m2")
            if out is None:
                out = sb.tile([P, T, 3], f32, tag=name)
            nc.vector.tensor_tensor(m1[:], a2[:, :, 1:4], b2[:, :, 2:5], Alu.mult)
            nc.vector.tensor_tensor(m2[:], a2[:, :, 2:5], b2[:, :, 1:4], Alu.mult)
            nc.vector.tensor_tensor(out[:], m1[:], m2[:], Alu.subtract)
            return out

        def hi_lo_stack(src, layout, name, eng=None):
            """src f32 [128,8,6] -> bf16 [128,8,18] stacked per `layout`
            ('h' = bf16 rounding of src, 'l' = residual src - hi)."""
            eng = eng or nc.vector
            out = sb.tile([P, T, 18], bf16, tag=name)
            hi_slot = layout.index('h') * 6
            eng.tensor_copy(out[:, :, hi_slot:hi_slot + 6], src[:])
            for g, kind in enumerate(layout):
                if g * 6 == hi_slot:
                    continue
                sl = out[:, :, g * 6:(g + 1) * 6]
                if kind == 'h':
                    eng.tensor_copy(sl, out[:, :, hi_slot:hi_slot + 6])
                else:
                    eng.tensor_tensor(
                        sl, src[:], out[:, :, hi_slot:hi_slot + 6], Alu.subtract)
            return out

        def hi_lo_stack_halves(src, layout, name, eng=None):
            """hi_lo_stack but emitted per t-half so the PE transposes of
            half 0 can start while half 1 is still stacking."""
            eng = eng or nc.vector
            out = sb.tile([P, T, 18], bf16, tag=name)
            hi_slot = layout.index('h') * 6
            for h in range(2):
                ts = slice(h * (T // 2), (h + 1) * (T // 2))
                eng.tensor_copy(out[:, ts, hi_slot:hi_slot + 6], src[:, ts])
                for g, kind in enumerate(layout):
                    if g * 6 == hi_slot:
                        continue
                    sl = out[:, ts, g * 6:(g + 1) * 6]
                    if kind == 'h':
                        eng.tensor_copy(sl, out[:, ts, hi_slot:hi_slot + 6])
                    else:
                        eng.tensor_tensor(sl, src[:, ts],
                                          out[:, ts, hi_slot:hi_slot + 6],
                                          Alu.subtract)
            return out

        # ---- frames -> Gram, sqrt-free.  The reference builds
        #   x = normalize(c-ca), v = normalize(n-ca), z = normalize(x X v),
        #   y = normalize(z X x),  G = xx^T + yy^T + zz^T.
        # Positive scale factors pass through cross products and cancel in
        # the direction, so with ux = c-ca, zr = cross(ux, n-ca),
        # yr = cross(zr, ux):
        #   xx^T = ux ux^T / ns_x,  zz^T = zr zr^T / ns_z,
        #   yy^T = yr yr^T / (ns_z * ns_x)        (||y_ref|| = 1 to ~1e-7)
        # with ns_* plain sums of squares -> the whole chain is DVE-only
        # (reciprocal instead of 1/(sqrt+eps); deviation ~1e-8 relative).
        u2 = sb.tile([P, 2, T, 3], f32)
        nc.vector.tensor_tensor(
            u2[:].rearrange("p a t c -> p t a c"), t_nc2,
            t_ca1.unsqueeze(2).broadcast_to([P, T, 2, 3]),
            Alu.subtract)
        # combined replicate of both u vectors: [128,2,8,3] -> [128,2,8,6]
        uu2 = sb.tile([P, 2, T, 6], f32)
        nc.vector.tensor_copy(
            uu2[:].rearrange("p a t (r c) -> p a t r c", r=2),
            u2[:].unsqueeze(3).broadcast_to([P, 2, T, 2, 3]),
        )
        uv2 = uu2[:, 0]
        ux2 = uu2[:, 1]
        zr = cross(ux2, uv2, "zr")
        zr2 = replicate(zr[:], "zr2")
        yr = cross(zr2, ux2, "yr")
        yr2 = replicate(yr[:], "yr2")
        sq2 = sb.tile([P, 2, T, 3], f32)
        nc.vector.tensor_tensor(sq2[:, 0], u2[:, 1], u2[:, 1], Alu.mult)
        nc.vector.tensor_tensor(sq2[:, 1], zr[:], zr[:], Alu.mult)
        ns2 = sb.tile([P, 2, T], f32)
        nc.vector.tensor_reduce(ns2[:], sq2[:], AxX, Alu.add)
        iv2 = sb.tile([P, 2, T], f32)
        nc.vector.reciprocal(iv2[:], ns2[:])
        ivzx = sb.tile([P, T], f32)
        nc.vector.tensor_tensor(ivzx[:], iv2[:, 0], iv2[:, 1], Alu.mult)

        # ---- w[i]: Gram of R_i, mask-folded.  P* = [diag(3) | offdiag(3)]
        def products(r2, name):
            out = sb.tile([P, T, 6], f32, tag=name)
            nc.vector.tensor_tensor(out[:, :, 0:3], r2[:, :, 0:3], r2[:, :, 0:3],
                                    Alu.mult)
            nc.vector.tensor_tensor(out[:, :, 3:6], r2[:, :, 0:3], r2[:, :, 1:4],
                                    Alu.mult)
            return out

        px = products(ux2, "px")
        pz = products(zr2, "pz")
        py = products(yr2, "py")
        bc6 = lambda v: v.unsqueeze(2).broadcast_to([P, T, 6])
        mx = sb.tile([P, T, 6], f32)
        mz = sb.tile([P, T, 6], f32)
        my = sb.tile([P, T, 6], f32)
        s1 = sb.tile([P, T, 6], f32)
        w_all = sb.tile([P, T, 6], f32)
        nc.vector.tensor_tensor(mx[:], px[:], bc6(iv2[:, 0]), Alu.mult)
        nc.vector.tensor_tensor(mz[:], pz[:], bc6(iv2[:, 1]), Alu.mult)
        nc.vector.tensor_tensor(my[:], py[:], bc6(ivzx[:]), Alu.mult)
        nc.vector.tensor_tensor(s1[:], mx[:], mz[:], Alu.add)
        nc.vector.tensor_tensor(w_all[:], s1[:], my[:], Alu.add)
        w_m = sb.tile([P, T, 6], f32)
        nc.vector.tensor_tensor(w_m[:], w_all[:], mask_bc6, Alu.mult)
        w18 = hi_lo_stack_halves(w_m, "hlh", "w18")  # rows [wh, wl, wh]

        # ---- q[j] path, entirely on Pool (gpsimd): keeps the DVE queue free
        # for the frame chain that gates the main loop.
        dd = sb.tile([P, T, 3], f32)
        nc.gpsimd.tensor_tensor(dd[:], t_pred, t_true, Alu.subtract)
        d2 = replicate(dd[:], "d2", eng=nc.gpsimd)
        q_all = sb.tile([P, T, 6], f32)
        nc.gpsimd.tensor_tensor(q_all[:, :, 0:3], dd[:], dd[:], Alu.mult)
        qc = sb.tile([P, T, 3], f32)
        nc.gpsimd.tensor_tensor(qc[:], d2[:, :, 0:3], d2[:, :, 1:4], Alu.mult)
        nc.gpsimd.tensor_tensor(q_all[:, :, 3:6], qc[:], qc[:], Alu.add)
        q_m = sb.tile([P, T, 6], f32)
        nc.gpsimd.tensor_tensor(q_m[:], q_all[:], mask_bc6, Alu.mult)
        q18 = hi_lo_stack(q_m, "hhl", "q18", eng=nc.gpsimd)  # rows [qh, qh, ql]

        # q transposes -> qT [18, 1024] bf16 (copies on ACT: keeps the DVE
        # queue free for the frame chain)
        qT = sb.tile([18, N], bf16)
        for half in range(2):
            pst = ps_t.tile([18, 4, P], bf16, tag="pst")
            for tt in range(4):
                t = half * 4 + tt
                nc.tensor.transpose(pst[:, tt, :], q18[:, t, :], ident[:])
            nc.scalar.copy(
                qT[:, half * 512:(half + 1) * 512],
                pst[:].rearrange("k f p -> k (f p)"),
            )

        # w transposes -> wT [18, 1024] bf16 (i' = transpose order; any
        # bijection of i is fine for the sum since mask is already folded in)
        wT = [sb.tile([18, 512], bf16, tag=f"wT{h}", name=f"wT{h}") for h in range(2)]
        for half in range(2):
            psw = ps_t.tile([18, 4, P], bf16, tag="pst")
            for tt in range(4):
                t = half * 4 + tt
                nc.tensor.transpose(psw[:, tt, :], w18[:, t, :], ident[:])
            nc.vector.tensor_copy(
                wT[half][:], psw[:].rearrange("k f p -> k (f p)"))

        # ---- main O(N^2) loop: per j-tile of 128, all 1024 i
        acc = sb.tile([P, 2 * T], f32)   # [sum_i diff | sum_i min(diff,10)]
        for t in range(1 if prep_only else T):
            nst = ps_ns.tile([P, N], f32, tag="nst")
            lhs = qT[:, t * P:(t + 1) * P]
            nc.tensor.matmul(nst[:, 0:512], lhs, wT[0][:], start=True, stop=True)
            nc.tensor.matmul(nst[:, 512:N], lhs, wT[1][:], start=True, stop=True)
            dft = dpool.tile([P, N], d_dt, tag="dft")
            nc.scalar.activation(dft[:], nst[:], Act.Sqrt)
            scr = spool.tile([P, N], d_dt, tag="scr")
            nc.vector.tensor_scalar(scr[:], dft[:], 10.0, 0.0, Alu.min, Alu.add,
                                    accum_out=acc[:, T + t:T + t + 1])
            scr2 = spool.tile([P, N], d_dt, tag="scr2")
            nc.vector.tensor_scalar(scr2[:], dft[:], 1.0, 0.0, Alu.mult, Alu.add,
                                    accum_out=acc[:, t:t + 1])

        # ---- per-core partial sums out; host reduces the 128x16 sheet.
        # Bulk columns go out while the last iteration still runs.
        oview = d_out.ap().rearrange("p (h t) -> p h t", h=2)
        aview = acc[:].rearrange("p (h t) -> p h t", h=2)
        nc.sync.dma_start(oview[:, :, 0:7], aview[:, :, 0:7])
        nc.sync.dma_start(oview[:, :, 7:8], aview[:, :, 7:8])

        if rep_ctx is not None:
            rep_ctx.__exit__(None, None, None)

    nc.compile()
    return nc


def _get_nc():
    if "nc" not in _cache:
        _cache["nc"] = _build_nc()
    return _cache["nc"]


def kernel(n, ca, c, pred_pos, true_pos, mask) -> np.ndarray:
    from concourse.bass_utils import run_bass_kernel_spmd

    nc = _get_nc()
    allc = np.concatenate(
        [np.asarray(n, np.float32), np.asarray(c, np.float32),
         np.asarray(ca, np.float32), np.asarray(pred_pos, np.float32),
         np.asarray(true_pos, np.float32),
         mask.astype(np.float32)[..., None]], axis=-1)
    allc = np.pad(allc, [(0, 0), (0, 0), (0, 16 - allc.shape[-1])])
    in_maps = [{"all_in": np.ascontiguousarray(allc[b])} for b in range(B)]
    res = run_bass_kernel_spmd(nc, in_maps, core_ids=list(range(NCORES)))
    total = float(sum(r["out_acc"].astype(np.float64).sum() for r in res.results))
    denom = float(mask.sum()) + 1e-8
    return np.float32(0.5 * total / denom)


# revision 33
# speedup vs baseline: 3.0312x; 3.0312x over previous
"""FAPE loss kernel for Trainium2 (Bass/Tile), 8 NeuronCores.

Problem: B=8, N=1024.  reference computes, per batch b:
    R_i, t_i = backbone frames from (n, ca, c)          [N,3,3],[N,3]
    diff[i,j] = || R_i^T (pred_j - t_i) - R_i^T (true_j - t_i) ||
    per_pair  = min(diff,10) + 0.5*(diff - min(diff,10)) = 0.5*(diff + min(diff,10))
    out = sum_b sum_ij m_i m_j per_pair / (sum(m) + 1e-8)

Key reassociation (exact, no orthonormality assumption):
    R_i^T (pred_j - t_i) - R_i^T (true_j - t_i) = R_i^T d_j,  d_j = pred_j - true_j
    diff^2[i,j] = d_j^T (R_i R_i^T) d_j = sum_k q_k[j] * w_k[i]   (K=6)
  with q[j] = [d0^2, d1^2, d2^2, 2*d0d1, 2*d1d2, 2*d2d0] (masked by m_j)
       w[i] = [G00, G11, G22, G01, G12, G20], G = R_i R_i^T (masked by m_i)

Pairwise O(N^2) part per j-tile of 128:
  - TensorEngine matmul, K=18 bf16 hi/lo split (q = qh+ql, w = wh+wl bf16;
    rows [qh,qh,ql] x [wh,wl,wh] -> q.w exact up to ~2^-18) -> normsq PSUM
  - ACT sqrt PSUM->SBUF bf16
  - two DVE tensor_scalar passes at 4x bf16 rate: min(diff,10) and copy,
    each with accum_out giving the per-row sums
Per-core output is the [128, 8+8] accumulator sheet (sum_i per j-row for
diff and clamped); host sums sheets from the 8 cores and normalizes.

Frame-build is sqrt-free (all deviations ~1e-7 relative, far below the
~2^-18 matmul and bf16-diff rounding this kernel already carries): since
x, y, z only enter G quadratically, their normalizations become reciprocal
scale factors on raw cross-product Grams,
    G = ux ux^T/ns_x + zr zr^T/ns_z + yr yr^T/(ns_x ns_z),
with ux = c-ca, zr = cross(ux, n-ca), yr = cross(zr, ux) — a pure-DVE
chain (no cross-engine sqrt round-trips on the critical path).

Sharding: batch-parallel, one batch per core (spec hint allows B data-parallel).
"""

import numpy as np

P = 128          # partitions
T = 8            # j = 8*p + t  (p-major; any index bijection works for the sum)
N = 1024
B = 8
NCORES = 8

_cache: dict = {}


def _build_nc(diff_dtype="bf16", reps=0, prep_only=False):
    """Emit the single-core BIR module (same NEFF runs SPMD on all 8 cores)."""
    from contextlib import ExitStack

    import concourse.bacc as bacc
    import concourse.mybir as mybir
    import concourse.tile as tile
    from concourse import masks
    from concourse._compat import axon_active

    f32 = mybir.dt.float32
    bf16 = mybir.dt.bfloat16
    d_dt = bf16 if diff_dtype == "bf16" else f32
    Alu = mybir.AluOpType
    Act = mybir.ActivationFunctionType
    AxX = mybir.AxisListType.X

    nc = bacc.Bacc(
        "TRN2",
        target_bir_lowering=False,
        debug=not axon_active(),
        num_devices=NCORES,
    )

    # One concatenated input: cols [n(3) c(3) ca(3) pred(3) true(3) mask(1)]
    d_all = nc.dram_tensor("all_in", [N, 16], f32, kind="ExternalInput")
    d_out = nc.dram_tensor("out_acc", [P, 2 * T], f32, kind="ExternalOutput")

    with tile.TileContext(nc) as tc, ExitStack() as ctx:
        sb = ctx.enter_context(tc.tile_pool(name="sb", bufs=1))
        ps_t = ctx.enter_context(tc.tile_pool(name="ps_t", bufs=2, space="PSUM"))
        ps_ns = ctx.enter_context(tc.tile_pool(name="ps_ns", bufs=2, space="PSUM"))
        dpool = ctx.enter_context(tc.tile_pool(name="dpool", bufs=3))
        spool = ctx.enter_context(tc.tile_pool(name="spool", bufs=3))

        # ---- ACT table warmup: force the sqrt set load early (overlaps DMA)
        warm = sb.tile([1, 2], f32)
        nc.vector.memset(warm[:], 1.0)
        nc.scalar.activation(warm[:, 1:2], warm[:, 0:1], Act.Sqrt)

        # ---- ONE input DMA: [1024,16] -> [128, 8, 16], j = 8*p + t.
        # Fully contiguous in DRAM, 512B per partition.  Issued first.
        stg = sb.tile([P, T, 16], f32)
        nc.sync.dma_start(stg[:], d_all.ap().rearrange("(p t) c -> p t c", p=P))

        ident = sb.tile([P, P], bf16)
        masks.make_identity(nc, ident[:])

        rep_ctx = tc.For_i(0, reps, 1) if reps else None
        if rep_ctx is not None:
            rep_ctx.__enter__()
        t_nc2 = stg[:, :, 0:6].rearrange("p t (a c) -> p t a c", a=2)
        t_ca1 = stg[:, :, 6:9]
        t_pred = stg[:, :, 9:12]
        t_true = stg[:, :, 12:15]
        mask_bc6 = stg[:, :, 15:16].broadcast_to([P, T, 6])

        # ---- helpers ------------------------------------------------------
        def replicate(vec, name, eng=None):
            """[128,8,3] view -> [128,8,6] with r2[:, t, c] = vec[:, t, c % 3]."""
            r2 = sb.tile([P, T, 6], f32, tag=name)
            (eng or nc.vector).tensor_copy(
                r2[:].rearrange("p t (r c) -> p t r c", r=2),
                vec.unsqueeze(2).broadcast_to([P, T, 2, 3]),
            )
            return r2

        def cross(a2, b2, name, out=None):
            m1 = sb.tile([P, T, 3], f32, tag=f"{name}_m1")
            m2 = sb.tile([P, T, 3], f32, tag=f"{name}_m2")
            if out is None:
                out = sb.tile([P, T, 3], f32, tag=name)
            nc.vector.tensor_tensor(m1[:], a2[:, :, 1:4], b2[:, :, 2:5], Alu.mult)
            nc.vector.tensor_tensor(m2[:], a2[:, :, 2:5], b2[:, :, 1:4], Alu.mult)
            nc.vector.tensor_tensor(out[:], m1[:], m2[:], Alu.subtract)
            return out

        def hi_lo_stack(src, layout, name, eng=None):
            """src f32 [128,8,6] -> bf16 [128,8,18] stacked per `layout`
            ('h' = bf16 rounding of src, 'l' = residual src - hi)."""
            eng = eng or nc.vector
            out = sb.tile([P, T, 18], bf16, tag=name)
            hi_slot = layout.index('h') * 6
            eng.tensor_copy(out[:, :, hi_slot:hi_slot + 6], src[:])
            for g, kind in enumerate(layout):
                if g * 6 == hi_slot:
                    continue
                sl = out[:, :, g * 6:(g + 1) * 6]
                if kind == 'h':
                    eng.tensor_copy(sl, out[:, :, hi_slot:hi_slot + 6])
                else:
                    eng.tensor_tensor(
                        sl, src[:], out[:, :, hi_slot:hi_slot + 6], Alu.subtract)
            return out

        def hi_lo_stack_halves(src, layout, name, eng=None):
            """hi_lo_stack but emitted per t-half so the PE transposes of
            half 0 can start while half 1 is still stacking."""
            eng = eng or nc.vector
            out = sb.tile([P, T, 18], bf16, tag=name)
            hi_slot = layout.index('h') * 6
            for h in range(2):
                ts = slice(h * (T // 2), (h + 1) * (T // 2))
                eng.tensor_copy(out[:, ts, hi_slot:hi_slot + 6], src[:, ts])
                for g, kind in enumerate(layout):
                    if g * 6 == hi_slot:
                        continue
                    sl = out[:, ts, g * 6:(g + 1) * 6]
                    if kind == 'h':
                        eng.tensor_copy(sl, out[:, ts, hi_slot:hi_slot + 6])
                    else:
                        eng.tensor_tensor(sl, src[:, ts],
                                          out[:, ts, hi_slot:hi_slot + 6],
                                          Alu.subtract)
            return out

        # ---- frames -> Gram, sqrt-free.  The reference builds
        #   x = normalize(c-ca), v = normalize(n-ca), z = normalize(x X v),
        #   y = normalize(z X x),  G = xx^T + yy^T + zz^T.
        # Positive scale factors pass through cross products and cancel in
        # the direction, so with ux = c-ca, zr = cross(ux, n-ca),
        # yr = cross(zr, ux):
        #   xx^T = ux ux^T / ns_x,  zz^T = zr zr^T / ns_z,
        #   yy^T = yr yr^T / (ns_z * ns_x)        (||y_ref|| = 1 to ~1e-7)
        # with ns_* plain sums of squares -> the whole chain is DVE-only
        # (reciprocal instead of 1/(sqrt+eps); deviation ~1e-8 relative).
        u2 = sb.tile([P, 2, T, 3], f32)
        nc.vector.tensor_tensor(
            u2[:].rearrange("p a t c -> p t a c"), t_nc2,
            t_ca1.unsqueeze(2).broadcast_to([P, T, 2, 3]),
            Alu.subtract)
        # combined replicate of both u vectors: [128,2,8,3] -> [128,2,8,6]
        uu2 = sb.tile([P, 2, T, 6], f32)
        nc.vector.tensor_copy(
            uu2[:].rearrange("p a t (r c) -> p a t r c", r=2),
            u2[:].unsqueeze(3).broadcast_to([P, 2, T, 2, 3]),
        )
        uv2 = uu2[:, 0]
        ux2 = uu2[:, 1]
        zr = cross(ux2, uv2, "zr")
        zr2 = replicate(zr[:], "zr2")
        yr = cross(zr2, ux2, "yr")
        yr2 = replicate(yr[:], "yr2")
        sq2 = sb.tile([P, 2, T, 3], f32)
        nc.vector.tensor_tensor(sq2[:, 0], u2[:, 1], u2[:, 1], Alu.mult)
        nc.vector.tensor_tensor(sq2[:, 1], zr[:], zr[:], Alu.mult)
        ns2 = sb.tile([P, 2, T], f32)
        nc.vector.tensor_reduce(ns2[:], sq2[:], AxX, Alu.add)
        iv2 = sb.tile([P, 2, T], f32)
        nc.vector.reciprocal(iv2[:], ns2[:])
        ivzx = sb.tile([P, T], f32)
        nc.vector.tensor_tensor(ivzx[:], iv2[:, 0], iv2[:, 1], Alu.mult)

        # ---- w[i]: Gram of R_i, mask-folded.  P* = [diag(3) | offdiag(3)]
        def products(r2, name):
            out = sb.tile([P, T, 6], f32, tag=name)
            nc.vector.tensor_tensor(out[:, :, 0:3], r2[:, :, 0:3], r2[:, :, 0:3],
                                    Alu.mult)
            nc.vector.tensor_tensor(out[:, :, 3:6], r2[:, :, 0:3], r2[:, :, 1:4],
                                    Alu.mult)
            return out

        px = products(ux2, "px")
        pz = products(zr2, "pz")
        py = products(yr2, "py")
        bc6 = lambda v: v.unsqueeze(2).broadcast_to([P, T, 6])
        mx = sb.tile([P, T, 6], f32)
        mz = sb.tile([P, T, 6], f32)
        my = sb.tile([P, T, 6], f32)
        s1 = sb.tile([P, T, 6], f32)
        w_all = sb.tile([P, T, 6], f32)
        nc.vector.tensor_tensor(mx[:], px[:], bc6(iv2[:, 0]), Alu.mult)
        nc.vector.tensor_tensor(mz[:], pz[:], bc6(iv2[:, 1]), Alu.mult)
        nc.vector.tensor_tensor(my[:], py[:], bc6(ivzx[:]), Alu.mult)
        nc.vector.tensor_tensor(s1[:], mx[:], mz[:], Alu.add)
        nc.vector.tensor_tensor(w_all[:], s1[:], my[:], Alu.add)
        w_m = sb.tile([P, T, 6], f32)
        nc.vector.tensor_tensor(w_m[:], w_all[:], mask_bc6, Alu.mult)
        w18 = hi_lo_stack_halves(w_m, "hlh", "w18")  # rows [wh, wl, wh]

        # ---- q[j] path, entirely on Pool (gpsimd): keeps the DVE queue free
        # for the frame chain that gates the main loop.
        dd = sb.tile([P, T, 3], f32)
        nc.gpsimd.tensor_tensor(dd[:], t_pred, t_true, Alu.subtract)
        d2 = replicate(dd[:], "d2", eng=nc.gpsimd)
        q_all = sb.tile([P, T, 6], f32)
        nc.gpsimd.tensor_tensor(q_all[:, :, 0:3], dd[:], dd[:], Alu.mult)
        qc = sb.tile([P, T, 3], f32)
        nc.gpsimd.tensor_tensor(qc[:], d2[:, :, 0:3], d2[:, :, 1:4], Alu.mult)
        nc.gpsimd.tensor_tensor(q_all[:, :, 3:6], qc[:], qc[:], Alu.add)
        q_m = sb.tile([P, T, 6], f32)
        nc.gpsimd.tensor_tensor(q_m[:], q_all[:], mask_bc6, Alu.mult)
        q18 = hi_lo_stack(q_m, "hhl", "q18", eng=nc.gpsimd)  # rows [qh, qh, ql]

        # q transposes -> qT [18, 1024] bf16 (copies on ACT: keeps the DVE
        # queue free for the frame chain)
        qT = sb.tile([18, N], bf16)
        for half in range(2):
            pst = ps_t.tile([18, 4, P], bf16, tag="pst")
            for tt in range(4):
                t = half * 4 + tt
                nc.tensor.transpose(pst[:, tt, :], q18[:, t, :], ident[:])
            nc.scalar.copy(
                qT[:, half * 512:(half + 1) * 512],
                pst[:].rearrange("k f p -> k (f p)"),
            )

        # w transposes -> wT [18, 1024] bf16 (i' = transpose order; any
        # bijection of i is fine for the sum since mask is already folded in)
        wT = [sb.tile([18, 512], bf16, tag=f"wT{h}", name=f"wT{h}") for h in range(2)]
        for half in range(2):
            psw = ps_t.tile([18, 4, P], bf16, tag="pst")
            for tt in range(4):
                t = half * 4 + tt
                nc.tensor.transpose(psw[:, tt, :], w18[:, t, :], ident[:])
            nc.vector.tensor_copy(
                wT[half][:], psw[:].rearrange("k f p -> k (f p)"))

        # ---- main O(N^2) loop: per j-tile of 128, all 1024 i
        acc = sb.tile([P, 2 * T], f32)   # [sum_i diff | sum_i min(diff,10)]
        for t in range(1 if prep_only else T):
            nst = ps_ns.tile([P, N], f32, tag="nst")
            lhs = qT[:, t * P:(t + 1) * P]
            nc.tensor.matmul(nst[:, 0:512], lhs, wT[0][:], start=True, stop=True)
            nc.tensor.matmul(nst[:, 512:N], lhs, wT[1][:], start=True, stop=True)
            dft = dpool.tile([P, N], d_dt, tag="dft")
            nc.scalar.activation(dft[:], nst[:], Act.Sqrt)
            scr = spool.tile([P, N], d_dt, tag="scr")
            nc.vector.tensor_scalar(scr[:], dft[:], 10.0, 0.0, Alu.min, Alu.add,
                                    accum_out=acc[:, T + t:T + t + 1])
            scr2 = spool.tile([P, N], d_dt, tag="scr2")
            nc.vector.tensor_scalar(scr2[:], dft[:], 1.0, 0.0, Alu.mult, Alu.add,
                                    accum_out=acc[:, t:t + 1])

        # ---- per-core partial sums out; host reduces the 128x16 sheet.
        # Bulk columns go out while the last iteration still runs.
        oview = d_out.ap().rearrange("p (h t) -> p h t", h=2)
        aview = acc[:].rearrange("p (h t) -> p h t", h=2)
        nc.sync.dma_start(oview[:, :, 0:7], aview[:, :, 0:7])
        nc.sync.dma_start(oview[:, :, 7:8], aview[:, :, 7:8])

        if rep_ctx is not None:
            rep_ctx.__exit__(None, None, None)

    nc.compile()
    return nc


def _get_nc():
    if "nc" not in _cache:
        _cache["nc"] = _build_nc()
    return _cache["nc"]


def kernel(n, ca, c, pred_pos, true_pos, mask) -> np.ndarray:
    from concourse.bass_utils import run_bass_kernel_spmd

    nc = _get_nc()
    allc = np.concatenate(
        [np.asarray(n, np.float32), np.asarray(c, np.float32),
         np.asarray(ca, np.float32), np.asarray(pred_pos, np.float32),
         np.asarray(true_pos, np.float32),
         mask.astype(np.float32)[..., None]], axis=-1)
    allc = np.pad(allc, [(0, 0), (0, 0), (0, 16 - allc.shape[-1])])
    in_maps = [{"all_in": np.ascontiguousarray(allc[b])} for b in range(B)]
    res = run_bass_kernel_spmd(nc, in_maps, core_ids=list(range(NCORES)))
    total = float(sum(r["out_acc"].astype(np.float64).sum() for r in res.results))
    denom = float(mask.sum()) + 1e-8
    return np.asarray(0.5 * total / denom, dtype=np.float32)
